# revision 5
# baseline (speedup 1.0000x reference)
"""Block-diagonal masked dense + BatchNorm(train) + ReLU on 8 TRN2 NeuronCores.

Math: out = x @ (W * blockdiag_mask) + bias; BN over batch; relu.
The mask keeps 64 diagonal blocks of shape [64 in, 64 out]. Group g only
couples x[:, 64g:64g+64] to out[:, 64g:64g+64].

Sharding: groups are split across cores (8 groups per core). Each core owns a
disjoint 512-column slice of both input and output features, so the matmul and
the per-feature batch statistics are fully core-local (no collectives).

Per-core device program (all shapes hardcoded):
  inputs:  xT [512, 4096] (x slice transposed on host), wd [512, 64] (stacked
           diagonal blocks), gm/bt [512] (gamma/beta slice)
  output:  yT [512, 4096] (y slice transposed; host transposes back)
  phase 1: for each 128-row chunk c (2 groups) and batch tile t (512):
           psum[j, b] = W_g^T x_g^T via 2 matmuls packed into PE quadrants
           (0,0)/(64,64); bn_stats/bn_aggr accumulate mean/var per feature.
  coefs:   A = gamma * rsqrt(var + eps); B = beta - mean * A.
           (bias cancels in BN: out and mean(out) shift equally, and variance
           is bias-invariant, so bias never needs to reach the device.)
  phase 2: recompute the matmul (x stays SBUF-resident) and apply
           relu(psum * A + B) in one ScalarE pass, PSUM -> SBUF -> DRAM.
"""

import numpy as np

import concourse.bass as bass
import concourse.tile as tile
from concourse import mybir
from concourse.bass_utils import run_bass_kernel_spmd

F32 = mybir.dt.float32

NCORES = 8
BATCH = 4096
DIM = 4096
DCORE = DIM // NCORES          # 512 features per core
CHUNKS = DCORE // 128          # 4 partition chunks (2 groups each)
BTILE = 512                    # batch tile (one PSUM bank, fp32 moving max)
BTILES = BATCH // BTILE        # 8
EPS = 1e-3

_MAX_WAITS = 1


def _split_multi_waits(nc: bass.Bass, max_waits: int = _MAX_WAITS) -> None:
    # The walrus build in this container rejects instructions carrying more
    # than one sync-wait command (any engine, any opcode). Hoist extra waits
    # onto same-engine NOPs inserted immediately before the instruction —
    # identical semantics, since the engine blocks on each wait in order.
    # Snapshot every block BEFORE creating any nop: the engine builders append
    # new instructions to the current (last) block as a side effect, and the
    # final wholesale reassignment below discards those spurious appends.
    snapshots = [
        (bb, list(bb.instructions)) for f in nc.m.functions for bb in f.blocks
    ]
    rebuilt = []
    for bb, insts in snapshots:
        new = []
        for ins in insts:
            si = getattr(ins, "sync_info", None)
            waits = list(si.on_wait) if si is not None and si.on_wait else []
            if len(waits) > max_waits:
                head = waits[:-max_waits]
                for i in range(0, len(head), max_waits):
                    nop = nc.engines[ins.engine].nop().ins
                    nop.sync_info = mybir.SyncInfo(
                        on_wait=head[i : i + max_waits], on_update=[]
                    )
                    new.append(nop)
                ins.sync_info = mybir.SyncInfo(
                    on_wait=waits[-max_waits:],
                    on_update=list(si.on_update or []),
                )
            new.append(ins)
        rebuilt.append((bb, new))
    for bb, new in rebuilt:
        bb.instructions = new


def _build_nc() -> bass.Bass:
    nc = bass.Bass()
    xT = nc.dram_tensor("xT", [DCORE, BATCH], F32, kind="ExternalInput")
    wd = nc.dram_tensor("wd", [DCORE, 64], F32, kind="ExternalInput")
    gm = nc.dram_tensor("gm", [DCORE], F32, kind="ExternalInput")
    bt = nc.dram_tensor("bt", [DCORE], F32, kind="ExternalInput")
    yT = nc.dram_tensor("yT", [DCORE, BATCH], F32, kind="ExternalOutput")

    with tile.TileContext(nc) as tc:
        with (
            tc.tile_pool(name="singles", bufs=1) as singles,
            tc.tile_pool(name="stats", bufs=1) as statp,
            tc.tile_pool(name="psum", bufs=8, space="PSUM") as psum,
            tc.tile_pool(name="y", bufs=6) as ypool,
        ):
            # Resident inputs. Partition p of chunk c holds feature c*128+p.
            xsb = singles.tile([128, CHUNKS, BATCH], F32)
            xTv = xT.rearrange("(c p) b -> p c b", p=128)
            for c in range(CHUNKS):
                for h in range(2):
                    sl = bass.ds(h * (BATCH // 2), BATCH // 2)
                    nc.sync.dma_start(xsb[:, c, sl], xTv[:, c, sl])
            wsb = singles.tile([128, CHUNKS, 64], F32)
            nc.sync.dma_start(wsb[:], wd.rearrange("(c p) m -> p c m", p=128))
            gsb = singles.tile([128, CHUNKS], F32)
            nc.sync.dma_start(gsb[:], gm.rearrange("(c p) -> p c", p=128))
            bsb = singles.tile([128, CHUNKS], F32)
            nc.sync.dma_start(bsb[:], bt.rearrange("(c p) -> p c", p=128))
            epsb = singles.tile([128, 1], F32)
            nc.vector.memset(epsb[:], EPS)

            stats = statp.tile([128, CHUNKS, BTILES, 6], F32)
            mv = statp.tile([128, CHUNKS, 2], F32)
            coefA = statp.tile([128, CHUNKS], F32)
            coefB = statp.tile([128, CHUNKS], F32)
            tmp = statp.tile([128, CHUNKS], F32)

            def group_pair_matmul(c: int, t: int):
                ps = psum.tile([128, BTILE], F32, tag="ps")
                bs = bass.ds(t * BTILE, BTILE)
                nc.tensor.matmul(
                    ps[0:64, :], lhsT=wsb[0:64, c, :], rhs=xsb[0:64, c, bs],
                    start=True, stop=True, tile_position=(0, 0),
                )
                nc.tensor.matmul(
                    ps[64:128, :], lhsT=wsb[64:128, c, :], rhs=xsb[64:128, c, bs],
                    start=True, stop=True, tile_position=(64, 64),
                )
                return ps

            # Phase 1: batch statistics per output feature.
            for c in range(CHUNKS):
                for t in range(BTILES):
                    ps = group_pair_matmul(c, t)
                    nc.vector.bn_stats(stats[:, c, t, :], ps[:, :])
                nc.vector.bn_aggr(mv[:, c, :], stats[:, c, :, :])
                nc.scalar.activation(
                    tmp[:, c : c + 1], mv[:, c, 1:2],
                    mybir.ActivationFunctionType.Sqrt,
                    bias=epsb[:], scale=1.0,
                )
                nc.vector.reciprocal(tmp[:, c : c + 1], tmp[:, c : c + 1])
                nc.vector.tensor_mul(
                    coefA[:, c : c + 1], tmp[:, c : c + 1], gsb[:, c : c + 1]
                )
                nc.vector.tensor_mul(
                    tmp[:, c : c + 1], mv[:, c, 0:1], coefA[:, c : c + 1]
                )
                nc.vector.tensor_sub(
                    coefB[:, c : c + 1], bsb[:, c : c + 1], tmp[:, c : c + 1]
                )

            # Phase 2: recompute matmul, fuse scale/shift/relu on ScalarE.
            yTv = yT.rearrange("(c p) b -> p c b", p=128)
            for c in range(CHUNKS):
                for t in range(BTILES):
                    ps = group_pair_matmul(c, t)
                    yt = ypool.tile([128, BTILE], F32, tag="yt")
                    nc.scalar.activation(
                        yt[:], ps[:],
                        mybir.ActivationFunctionType.Relu,
                        bias=coefB[:, c : c + 1], scale=coefA[:, c : c + 1],
                    )
                    nc.sync.dma_start(yTv[:, c, bass.ds(t * BTILE, BTILE)], yt[:])
    _split_multi_waits(nc)
    return nc


_NC_CACHE: bass.Bass | None = None


def _get_nc() -> bass.Bass:
    global _NC_CACHE
    if _NC_CACHE is None:
        _NC_CACHE = _build_nc()
    return _NC_CACHE


def _make_in_maps(x, weight, gamma, beta):
    in_maps = []
    for c in range(NCORES):
        sl = slice(c * DCORE, (c + 1) * DCORE)
        xT = np.ascontiguousarray(x[:, sl].T)
        wdc = np.empty((DCORE, 64), np.float32)
        for g in range(DCORE // 64):
            r = slice(c * DCORE + g * 64, c * DCORE + (g + 1) * 64)
            wdc[g * 64 : (g + 1) * 64, :] = weight[r, r]
        in_maps.append(
            {
                "xT": xT,
                "wd": wdc,
                "gm": np.ascontiguousarray(gamma[sl]),
                "bt": np.ascontiguousarray(beta[sl]),
            }
        )
    return in_maps


def kernel(x, weight, bias, gamma, beta, **_run_kwargs) -> np.ndarray:
    x = np.asarray(x, np.float32)
    weight = np.asarray(weight, np.float32)
    gamma = np.asarray(gamma, np.float32)
    beta = np.asarray(beta, np.float32)
    # bias is algebraically irrelevant: BN subtracts the batch mean, which
    # absorbs any constant per-feature shift, and variance is shift-invariant.

    nc = _get_nc()
    res = run_bass_kernel_spmd(
        nc, _make_in_maps(x, weight, gamma, beta),
        core_ids=list(range(NCORES)), **_run_kwargs,
    )
    out = np.empty((BATCH, DIM), np.float32)
    for c, r in enumerate(res.results):
        out[:, c * DCORE : (c + 1) * DCORE] = r["yT"].T
    kernel.last_results = res
    return out


# revision 11
# speedup vs baseline: 1.5001x; 1.5001x over previous
"""Block-diagonal masked dense + BatchNorm(train) + ReLU on 8 TRN2 NeuronCores.

Math: out = x @ (W * blockdiag_mask) + bias; BN over batch; relu.
The mask keeps 64 diagonal blocks of shape [64 in, 64 out]. Group g only
couples x[:, 64g:64g+64] to out[:, 64g:64g+64].

Sharding: groups are split across cores (8 groups per core). Each core owns a
disjoint 512-column slice of both input and output features, so the matmul and
the per-feature batch statistics are fully core-local (no collectives).

Per-core device program (all shapes hardcoded):
  inputs:  xT [512, 4096] (x slice transposed on host), wd [512, 64] (stacked
           diagonal blocks), gm/bt [512] (gamma/beta slice)
  output:  yT [512, 4096] (y slice transposed; host transposes back)
  phase 1: for each 128-row chunk c (2 groups) and batch tile t (512):
           psum[j, b] = W_g^T x_g^T via 2 matmuls packed into PE quadrants
           (0,0)/(64,64); bn_stats/bn_aggr accumulate mean/var per feature.
  coefs:   A = gamma * rsqrt(var + eps); B = beta - mean * A.
           (bias cancels in BN: out and mean(out) shift equally, and variance
           is bias-invariant, so bias never needs to reach the device.)
  phase 2: recompute the matmul (x stays SBUF-resident) and apply
           relu(psum * A + B) in one ScalarE pass, PSUM -> SBUF -> DRAM.
"""

import numpy as np

import concourse.bass as bass
import concourse.tile as tile
from concourse import mybir
from concourse.bass_utils import run_bass_kernel_spmd

F32 = mybir.dt.float32

NCORES = 8
BATCH = 4096
DIM = 4096
DCORE = DIM // NCORES          # 512 features per core
CHUNKS = DCORE // 128          # 4 partition chunks (2 groups each)
BTILE = 512                    # batch tile (one PSUM bank, fp32 moving max)
BTILES = BATCH // BTILE        # 8
EPS = 1e-3

_MAX_WAITS = 1


def _split_multi_waits(nc: bass.Bass, max_waits: int = _MAX_WAITS) -> None:
    # The walrus build in this container rejects instructions carrying more
    # than one sync-wait command (any engine, any opcode). Hoist extra waits
    # onto same-engine NOPs inserted immediately before the instruction —
    # identical semantics, since the engine blocks on each wait in order.
    # Snapshot every block BEFORE creating any nop: the engine builders append
    # new instructions to the current (last) block as a side effect, and the
    # final wholesale reassignment below discards those spurious appends.
    snapshots = [
        (bb, list(bb.instructions)) for f in nc.m.functions for bb in f.blocks
    ]
    rebuilt = []
    for bb, insts in snapshots:
        new = []
        for ins in insts:
            si = getattr(ins, "sync_info", None)
            waits = list(si.on_wait) if si is not None and si.on_wait else []
            if len(waits) > max_waits:
                head = waits[:-max_waits]
                for i in range(0, len(head), max_waits):
                    nop = nc.engines[ins.engine].nop().ins
                    nop.sync_info = mybir.SyncInfo(
                        on_wait=head[i : i + max_waits], on_update=[]
                    )
                    new.append(nop)
                ins.sync_info = mybir.SyncInfo(
                    on_wait=waits[-max_waits:],
                    on_update=list(si.on_update or []),
                )
            new.append(ins)
        rebuilt.append((bb, new))
    for bb, new in rebuilt:
        bb.instructions = new


F32R = mybir.dt.float32r
MEGA = 1024                    # PSUM mega-tile free dim (2 banks, 2 matmuls)
MEGAS = BATCH // MEGA          # 4 mega tiles per chunk per phase


def _build_nc() -> bass.Bass:
    nc = bass.Bass()
    # x and the diagonal weight blocks arrive pre-rounded to float32r's
    # 11-bit mantissa (host-side), so the f32r matmul is exact on them and
    # the PE streams at 1 cycle/row instead of fp32's ~4.
    xT = nc.dram_tensor("xT", [DCORE, BATCH], F32, kind="ExternalInput")
    wd = nc.dram_tensor("wd", [DCORE, 128], F32, kind="ExternalInput")
    gm = nc.dram_tensor("gm", [DCORE], F32, kind="ExternalInput")
    bt = nc.dram_tensor("bt", [DCORE], F32, kind="ExternalInput")
    yT = nc.dram_tensor("yT", [DCORE, BATCH], F32, kind="ExternalOutput")

    with tile.TileContext(nc) as tc:
        with (
            tc.tile_pool(name="singles", bufs=1) as singles,
            tc.tile_pool(name="stats", bufs=1) as statp,
            tc.tile_pool(name="psum", bufs=4, space="PSUM") as psum,
            tc.tile_pool(name="y", bufs=4) as ypool,
        ):
            # Small operands first: every matmul self-loads weights, so wd
            # must not queue behind 8 MB of x on the DMA ring.
            wsb = singles.tile([128, CHUNKS, 128], F32R)
            nc.sync.dma_start(
                wsb[:], wd.rearrange("(c p) m -> p c m", p=128).bitcast(F32R)
            )
            gsb = singles.tile([128, CHUNKS], F32)
            nc.sync.dma_start(gsb[:], gm.rearrange("(c p) -> p c", p=128))
            bsb = singles.tile([128, CHUNKS], F32)
            nc.sync.dma_start(bsb[:], bt.rearrange("(c p) -> p c", p=128))
            epsb = singles.tile([128, 1], F32)
            nc.vector.memset(epsb[:], EPS)

            # Resident x. Partition p of chunk c holds feature c*128+p.
            # Chunk 0 lands in quarters so compute starts ASAP.
            xsb = singles.tile([128, CHUNKS, BATCH], F32R)
            xTv = xT.rearrange("(c p) b -> p c b", p=128).bitcast(F32R)
            for c in range(CHUNKS):
                nq = 4 if c == 0 else 2
                step = BATCH // nq
                for h in range(nq):
                    sl = bass.ds(h * step, step)
                    nc.sync.dma_start(xsb[:, c, sl], xTv[:, c, sl])

            stats = statp.tile([128, CHUNKS, BTILES, 6], F32)
            mv = statp.tile([128, CHUNKS, 2], F32)
            coefA = statp.tile([128, CHUNKS], F32)
            coefB = statp.tile([128, CHUNKS], F32)
            tmp = statp.tile([128, CHUNKS], F32)

            def mega_matmul(c: int, m: int):
                # One [128, 1024] PSUM tile = 2 banks = 2 batch tiles. Each
                # matmul contracts K=128 against a 2x2 block-diagonal
                # stationary (two 64x64 group blocks; zeros kill the cross
                # terms), so the output spans the full 128 partitions.
                ps = psum.tile([128, MEGA], F32, tag="ps")
                for s in range(MEGA // BTILE):
                    t = m * (MEGA // BTILE) + s
                    nc.tensor.matmul(
                        ps[:, bass.ds(s * BTILE, BTILE)],
                        lhsT=wsb[:, c, :],
                        rhs=xsb[:, c, bass.ds(t * BTILE, BTILE)],
                        start=True, stop=True,
                    )
                return ps

            def phase1_chunk(c: int):
                for m in range(MEGAS):
                    ps = mega_matmul(c, m)
                    for s in range(MEGA // BTILE):
                        t = m * (MEGA // BTILE) + s
                        nc.vector.bn_stats(
                            stats[:, c, t, :], ps[:, bass.ds(s * BTILE, BTILE)]
                        )
                nc.vector.bn_aggr(mv[:, c, :], stats[:, c, :, :])
                nc.scalar.activation(
                    tmp[:, c : c + 1], mv[:, c, 1:2],
                    mybir.ActivationFunctionType.Sqrt,
                    bias=epsb[:], scale=1.0,
                )
                nc.vector.reciprocal(tmp[:, c : c + 1], tmp[:, c : c + 1])
                nc.vector.tensor_mul(
                    coefA[:, c : c + 1], tmp[:, c : c + 1], gsb[:, c : c + 1]
                )
                nc.vector.tensor_mul(
                    tmp[:, c : c + 1], mv[:, c, 0:1], coefA[:, c : c + 1]
                )
                nc.vector.tensor_sub(
                    coefB[:, c : c + 1], bsb[:, c : c + 1], tmp[:, c : c + 1]
                )

            yTv = yT.rearrange("(c p) b -> p c b", p=128)

            def phase2_chunk(c: int):
                for m in range(MEGAS):
                    ps = mega_matmul(c, m)
                    yt = ypool.tile([128, MEGA], F32, tag="yt")
                    nc.scalar.activation(
                        yt[:], ps[:],
                        mybir.ActivationFunctionType.Relu,
                        bias=coefB[:, c : c + 1], scale=coefA[:, c : c + 1],
                    )
                    nc.sync.dma_start(yTv[:, c, bass.ds(m * MEGA, MEGA)], yt[:])

            # Interleave so DVE (phase-1 stats) and ACT (phase-2 relu) work
            # concurrently instead of back-to-back, and output DMA starts
            # while input DMA is still streaming.
            phase1_chunk(0)
            phase1_chunk(1)
            phase2_chunk(0)
            phase1_chunk(2)
            phase2_chunk(1)
            phase1_chunk(3)
            phase2_chunk(2)
            phase2_chunk(3)
    _split_multi_waits(nc)
    return nc


_NC_CACHE: bass.Bass | None = None


def _get_nc() -> bass.Bass:
    global _NC_CACHE
    if _NC_CACHE is None:
        _NC_CACHE = _build_nc()
    return _NC_CACHE


def _round_f32r(a: np.ndarray) -> np.ndarray:
    # float32r keeps an 11-bit mantissa (HW rounds half-up; verified on
    # device). Pre-rounding on the host makes the device data a fixed point
    # of that rounding, so no on-device rounding pass is needed.
    ai = np.ascontiguousarray(a).view(np.uint32)
    out = ((ai.astype(np.uint64) + 0x800) & 0xFFFFF000).astype(np.uint32)
    return out.view(np.float32).reshape(a.shape)


def _make_in_maps(x, weight, gamma, beta):
    in_maps = []
    for c in range(NCORES):
        sl = slice(c * DCORE, (c + 1) * DCORE)
        xT = _round_f32r(np.ascontiguousarray(x[:, sl].T))
        # Per 128-row chunk: [[w_{2c}, 0], [0, w_{2c+1}]] block-diagonal.
        wdc = np.zeros((DCORE, 128), np.float32)
        for g in range(DCORE // 64):
            r = slice(c * DCORE + g * 64, c * DCORE + (g + 1) * 64)
            col = (g % 2) * 64
            wdc[g * 64 : (g + 1) * 64, col : col + 64] = weight[r, r]
        in_maps.append(
            {
                "xT": xT,
                "wd": _round_f32r(wdc),
                "gm": np.ascontiguousarray(gamma[sl]),
                "bt": np.ascontiguousarray(beta[sl]),
            }
        )
    return in_maps


def kernel(x, weight, bias, gamma, beta, **_run_kwargs) -> np.ndarray:
    x = np.asarray(x, np.float32)
    weight = np.asarray(weight, np.float32)
    gamma = np.asarray(gamma, np.float32)
    beta = np.asarray(beta, np.float32)
    # bias is algebraically irrelevant: BN subtracts the batch mean, which
    # absorbs any constant per-feature shift, and variance is shift-invariant.

    nc = _get_nc()
    res = run_bass_kernel_spmd(
        nc, _make_in_maps(x, weight, gamma, beta),
        core_ids=list(range(NCORES)), **_run_kwargs,
    )
    out = np.empty((BATCH, DIM), np.float32)
    for c, r in enumerate(res.results):
        out[:, c * DCORE : (c + 1) * DCORE] = r["yT"].T
    kernel.last_results = res
    return out
